# revision 40
# baseline (speedup 1.0000x reference)
"""ActiveShiftLayer Trainium2 kernel.

out[n,c,h,w] = bilinear sample of x[n,c, h+alpha_c, w+beta_c], zero outside
the spatial extent.

alpha,beta in [-1,1) => floor in {-1,0}, so the bilinear sample is a
separable 3-tap convolution along H then W with per-channel tap weights;
per channel only 2 of the 3 taps are nonzero in each direction.

Design (measured 63.8-67us HW exec on 8 trn2 cores; v1 baseline was 83.5us):
- fp16 input: x is cast to fp16 on the host, halving HBM read traffic
  (error budget 2e-2 >> fp16's ~1e-4). Output is stored fp16 and upcast on
  the host. ~12.9 MB of DMA per core.
- channel sort: channels are permuted on the host, grouped by floor(alpha)
  (secondary floor(beta)). A 128-channel block whose channels share
  floor(alpha) needs only 2 V-tap matmul passes instead of 3 (the one
  mixed block needs 3).
- V-stage on TensorE: accumulating fp16 diag-stationary matmuls into PSUM
  (2048-col pieces = 4 banks, 512-col chunks); ScalarE drains PSUM -> VT
  (fp16 SBUF, 1-element guards) - its only elementwise pass (~0.96
  ns/elem, the PSUM read rate).
- H-stage for 6 of 8 tiles entirely on VectorE in fp16 fast paths:
  OUT = VT*wh0 and TMP_L/R = VT*wh_m1/p1 are tensor_scalar ops (4x_2p,
  ~0.3 ns/elem); the two shifted accumulates are tensor_tensor adds
  (2x_1p) whose 3D access patterns exclude the out-of-range column.
  TMP_L/R are exactly zero on other-side channels so full-partition-range
  adds are no-ops there. scalar_tensor_tensor was rejected: it supports
  NO DVE perf modes (1 elem/cycle).
- H-stage for tiles {5,7} on TensorE (PE-H): 3 flat-tap diag matmuls over
  the guarded VT; the wrapped edge columns are corrected by tiny strided
  STTs. Tile 7 runs a per-piece lagged fixup+store epilogue to shrink the
  pipeline drain. This balances PE (~38us) against DVE (~41us), the two
  pacing engines.
- The PE-H H-phase is emitted one tile late (software pipelining) so it
  never delays the next tile's V matmuls.
- queues: loads + DVE-tile stores on sync HWDGE (loads are prefetched 4
  tiles ahead so store sem-waits never starve the PE); PE-H piece stores
  on GpSimd SWDGE. Stores must NOT share the Scalar queue: a store
  waiting on DVE head-of-line blocks the next PSUM drain.
- X and VT live in persistent manually-rotated buffers so zero guards are
  memset once; tile 0's H-stage runs in 3 row segments to start the DVE
  stream at ~6us instead of ~10us.

Rejected experiments (measured worse): GpSimd tensor_tensor offload
(Add runs at ~0.42 efficiency), 3+ PE-H tiles (PE overloads), partial
row-split PE-H, ACT-issued center scale, 32-aligned partition-split STT
from PSUM (DVE time depends only on free size, so partition splitting
saves nothing).

The Bass program is built after seeing shift_param (tap sets / partition
runs are data-dependent) and cached by that structure.

Sharding: data-parallel over batch (N=32 -> 4 per core), each core also
splits C=256 into two partition blocks -> 8 tiles of [128 channels
(partitions), 56x56 plane (free dim)] per core. Pure SPMD, no collectives.
"""

import os
import numpy as np

N, C, H, W = 32, 256, 56, 56
NCORES = 8
NSH = N // NCORES  # batches per core
P = 128
CB = C // P        # channel blocks
HW = H * W         # 3136
CHUNK = 512        # one PSUM bank of f32 per matmul
XLEN = W + HW + W + 16  # guard row above/below + pad
# rows of each shifted add handled by Pool (GpSimd) instead of DVE
POOLROWS = int(os.environ.get("ASL_POOLROWS", "0"))
# tile -> rows of its H-stage to run on TensorE (partial PE-H)
PARTROWS = {}
for _kv in os.environ.get("ASL_PARTROWS", "").split(","):
    if _kv and ":" in _kv:
        _k, _v = _kv.split(":")
        if int(_v):
            PARTROWS[int(_k)] = int(_v)

_CACHE = {}


def _build_nc(taps, runs, peh):
    """taps: per-cb tuple of dy offsets needed (subset of (-1,0,1)).
    runs: per-cb tuple of (p0, p1, side) partition ranges; side 0 = left
    tap (out[w] += tmp[w-1]), side 1 = right tap. peh: tile indices whose
    H-stage runs on TensorE (flat taps over guarded VT + wrap fixups)."""
    import concourse.bacc as bacc
    import concourse.mybir as mybir
    import concourse.tile as tile

    f32 = mybir.dt.float32
    f16 = mybir.dt.float16
    add = mybir.AluOpType.add
    mult = mybir.AluOpType.mult
    act_copy = mybir.ActivationFunctionType.Copy

    nc = bacc.Bacc()
    xs = nc.dram_tensor("xs", [NSH, C, H, W], f16, kind="ExternalInput")
    NT0 = max(len(t) for t in taps)
    # V-tap diags followed by 3 H-tap diags (wh_m1, wh_0, wh_p1)
    wd = nc.dram_tensor("wd", [CB, P, (NT0 + 3) * P], f16,
                        kind="ExternalInput")
    # wv[cb] columns: [wh_0, wh_m1, wh_p1, -wh_m1, -wh_p1, wh_m1+wh_p1]
    wv = nc.dram_tensor("wv", [CB, P, 6], f32, kind="ExternalInput")
    ys = nc.dram_tensor("ys", [NSH, C, H, W], f16, kind="ExternalOutput")

    with tile.TileContext(nc) as tc:
        with tc.tile_pool(name="wp", bufs=1) as wp, \
             tc.tile_pool(name="vt", bufs=3) as vpool, \
             tc.tile_pool(name="tm", bufs=4) as tpool, \
             tc.tile_pool(name="op", bufs=4) as opool, \
             tc.tile_pool(name="ps", bufs=2, space="PSUM") as ppool:

            wdt = []
            wvt = []

            def load_weights(cb):
                t = wp.tile([P, (NT0 + 3) * P], f16, tag=f"wd{cb}")
                nc.sync.dma_start(t[:], wd[cb])
                wdt.append(t)
                v = wp.tile([P, 6], f32, tag=f"wv{cb}")
                nc.sync.dma_start(v[:], wv[cb])
                wvt.append(v)

            # persistent X buffers: guards zeroed once, loads only rewrite
            # the middle region
            NXBUF = 5
            xbufs = []
            for i in range(NXBUF):
                xb = wp.tile([P, XLEN], f16, tag=f"X{i}")
                nc.vector.memset(xb[:, 0:W], 0.0)
                nc.vector.memset(xb[:, W + HW:W + HW + W], 0.0)
                xbufs.append(xb)
            # persistent VT buffers with 1-element guards at 0 and 1+HW
            # for the PE H-stage flat taps
            NVBUF = 4
            vbufs = []
            for i in range(NVBUF):
                vb = wp.tile([P, 3200], f16, tag=f"V{i}")
                nc.vector.memset(vb[:, 0:1], 0.0)
                nc.vector.memset(vb[:, 1 + HW:2 + HW], 0.0)
                vbufs.append(vb)

            load_weights(0)

            tiles = [(n, cb) for n in range(NSH) for cb in range(CB)]
            NT = len(tiles)

            # column boundaries of PSUM pieces per tile (<= 2048 cols each)
            def bounds(idx):
                if idx == 0:
                    return [0, 512, 1024, 2048, 3136]
                if idx == NT - 1:
                    # row-aligned so the PE-H epilogue can run per piece
                    return [0, 1008, 2016, 2576, 3136]
                return [0, 2048, 3136]

            def issue_load(idx):
                # segmented so piece i only depends on segments 0..i
                ln, lcb = tiles[idx]
                lcs = slice(lcb * P, (lcb + 1) * P)
                X = xbufs[idx % NXBUF]
                xflat = xs[ln, lcs, :, :].rearrange("p h w -> p (h w)")
                b = bounds(idx)
                cuts = [min(-(-c1 // W) + 1, H) for c1 in b[1:-1]] + [H]
                r0 = 0
                for r1 in cuts:
                    if r1 > r0:
                        nc.sync.dma_start(X[:, W + r0 * W:W + r1 * W],
                                          xflat[:, r0 * W:r1 * W])
                    r0 = r1
                return X

            xtiles = {}
            xtiles[0] = issue_load(0)
            load_weights(1)
            xtiles[1] = issue_load(1)
            xtiles[2] = issue_load(2)
            xtiles[3] = issue_load(3)

            pend = []  # (tidx, n, cb, Vb, OUT) awaiting H-phase

            def emit_h(tidx, n, cb, Vb, OUT):
                wvc = wvt[cb]
                cs = slice(cb * P, (cb + 1) * P)
                VT = Vb[:, 1:1 + HW]
                tb = bounds(tidx)
                o3 = OUT[:].rearrange("p (h w) -> p h w", w=W)
                hsegs = ([(0, 9), (9, 28), (28, 56)] if tidx == 0 else
                         [(0, 56)])
                if tidx in peh:
                    # H-stage on TensorE: 3 flat taps over guarded VT;
                    # wrapped columns corrected by two tiny STTs after.
                    # For the final tile the fixup+store epilogue runs per
                    # piece (lagged one piece so the vg55 read of the next
                    # row's vt is drained), shrinking the pipeline tail.
                    vg0 = Vb[:, 0:HW].rearrange(
                        "p (h w) -> p h w", w=W)[:, :, 0]
                    vg55 = Vb[:, 57:57 + HW].rearrange(
                        "p (h w) -> p h w", w=W)[:, :, 0]
                    yflatp = ys[n, cs, :, :].rearrange("p h w -> p (h w)")

                    def fix_store(r0, r1):
                        nc.vector.scalar_tensor_tensor(
                            o3[:, r0:r1, 0], vg0[:, r0:r1], wvc[:, 3:4],
                            o3[:, r0:r1, 0], op0=mult, op1=add)
                        nc.vector.scalar_tensor_tensor(
                            o3[:, r0:r1, W - 1], vg55[:, r0:r1],
                            wvc[:, 4:5], o3[:, r0:r1, W - 1],
                            op0=mult, op1=add)
                        nc.gpsimd.dma_start(yflatp[:, r0 * W:r1 * W],
                                            OUT[:, r0 * W:r1 * W])

                    lagged = tidx == NT - 1
                    prev = None
                    for c0, c1 in zip(tb[:-1], tb[1:]):
                        PZ = c1 - c0
                        PS2 = ppool.tile([P, 4 * CHUNK], f32, tag="ps")
                        for ti in range(3):
                            for k0 in range(0, PZ, CHUNK):
                                cn = min(CHUNK, PZ - k0)
                                o = c0 + k0 + ti  # dx = ti - 1, +1 guard
                                nc.tensor.matmul(
                                    PS2[:, k0:k0 + cn],
                                    wdt[cb][:, (NT0 + ti) * P:
                                             (NT0 + ti + 1) * P],
                                    Vb[:, o:o + cn],
                                    start=(ti == 0), stop=(ti == 2))
                        nc.scalar.activation(OUT[:, c0:c1], PS2[:, 0:PZ],
                                             act_copy)
                        if lagged:
                            if prev is not None:
                                fix_store(prev[0] // W, prev[1] // W)
                            prev = (c0, c1)
                    if lagged:
                        fix_store(prev[0] // W, prev[1] // W)
                    else:
                        fix_store(0, H)
                    return
                # H-stage on DVE in fp16 fast paths (tensor_scalar at
                # 4x_2p, shifted tensor_tensor adds at 2x_1p).
                # Separate prescaled planes per side: TMP_L/TMP_R are
                # exactly zero on other-side channels, so each
                # full-range shifted add is a no-op there.
                actc = tidx in (2, 4) and int(
                    os.environ.get("ASL_ACTC", "0"))
                # partial PE-H: rows [0, pr) of this tile's H-stage run on
                # TensorE (flat taps + fixups), the rest stays on DVE
                pr = PARTROWS.get(tidx, 0)
                if pr:
                    pcc = pr * W
                    vg0 = Vb[:, 0:HW].rearrange(
                        "p (h w) -> p h w", w=W)[:, :, 0]
                    vg55 = Vb[:, 57:57 + HW].rearrange(
                        "p (h w) -> p h w", w=W)[:, :, 0]
                    yflatp = ys[n, cs, :, :].rearrange("p h w -> p (h w)")
                    PS2 = ppool.tile([P, 4 * CHUNK], f32, tag="ps")
                    for ti in range(3):
                        for k0 in range(0, pcc, CHUNK):
                            cn = min(CHUNK, pcc - k0)
                            nc.tensor.matmul(
                                PS2[:, k0:k0 + cn],
                                wdt[cb][:, (NT0 + ti) * P:
                                         (NT0 + ti + 1) * P],
                                Vb[:, k0 + ti:k0 + ti + cn],
                                start=(ti == 0), stop=(ti == 2))
                    nc.scalar.activation(OUT[:, 0:pcc], PS2[:, 0:pcc],
                                         act_copy)
                    nc.vector.scalar_tensor_tensor(
                        o3[:, 0:pr, 0], vg0[:, 0:pr], wvc[:, 3:4],
                        o3[:, 0:pr, 0], op0=mult, op1=add)
                    nc.vector.scalar_tensor_tensor(
                        o3[:, 0:pr, W - 1], vg55[:, 0:pr],
                        wvc[:, 4:5], o3[:, 0:pr, W - 1],
                        op0=mult, op1=add)
                    nc.gpsimd.dma_start(yflatp[:, 0:pcc], OUT[:, 0:pcc])
                    hsegs = [(pr, 56)]
                for (r0, r1) in hsegs:
                    q0, q1 = r0 * W, r1 * W
                    if actc:
                        # offload the center scale to ScalarE (has slack)
                        nc.scalar.activation(OUT[:, q0:q1], VT[:, q0:q1],
                                             act_copy, scale=wvc[:, 0:1])
                    else:
                        nc.vector.tensor_scalar_mul(
                            OUT[:, q0:q1], VT[:, q0:q1], wvc[:, 0:1])
                    single, rblk = runs[cb]
                    t3s = {}
                    if single:
                        # one TMP with the per-channel nonzero outer
                        # weight; the adds use exact partition runs
                        TMP = tpool.tile([P, HW], f16, tag="tmp0")
                        nc.vector.tensor_scalar_mul(
                            TMP[:, q0:q1], VT[:, q0:q1], wvc[:, 5:6])
                        t3s[0] = t3s[1] = TMP[:].rearrange(
                            "p (h w) -> p h w", w=W)
                    else:
                        for (_, _, s) in rblk:
                            TMP = tpool.tile([P, HW], f16, tag=f"tmp{s}")
                            nc.vector.tensor_scalar_mul(
                                TMP[:, q0:q1], VT[:, q0:q1],
                                wvc[:, 1 + s:2 + s])
                            t3s[s] = TMP[:].rearrange(
                                "p (h w) -> p h w", w=W)
                    hs = min(POOLROWS, r1 - r0) if r0 == 0 else 0
                    for (a, b2, side) in rblk:
                        t3 = t3s[side]
                        if side == 0:
                            if hs:
                                nc.gpsimd.tensor_tensor(
                                    o3[a:b2, r0:r0 + hs, 1:W],
                                    t3[a:b2, r0:r0 + hs, 0:W - 1],
                                    o3[a:b2, r0:r0 + hs, 1:W], op=add)
                            nc.vector.tensor_tensor(
                                o3[a:b2, r0 + hs:r1, 1:W],
                                t3[a:b2, r0 + hs:r1, 0:W - 1],
                                o3[a:b2, r0 + hs:r1, 1:W], op=add)
                        else:
                            if hs:
                                nc.gpsimd.tensor_tensor(
                                    o3[a:b2, r0:r0 + hs, 0:W - 1],
                                    t3[a:b2, r0:r0 + hs, 1:W],
                                    o3[a:b2, r0:r0 + hs, 0:W - 1], op=add)
                            nc.vector.tensor_tensor(
                                o3[a:b2, r0 + hs:r1, 0:W - 1],
                                t3[a:b2, r0 + hs:r1, 1:W],
                                o3[a:b2, r0 + hs:r1, 0:W - 1], op=add)
                    yflat = ys[n, cs, :, :].rearrange("p h w -> p (h w)")
                    nc.sync.dma_start(yflat[:, q0:q1], OUT[:, q0:q1])

            for tidx, (n, cb) in enumerate(tiles):
                cs = slice(cb * P, (cb + 1) * P)
                if tidx + 4 < NT:
                    xtiles[tidx + 4] = issue_load(tidx + 4)
                X = xtiles.pop(tidx)

                Vb = vbufs[tidx % NVBUF]
                OUT = opool.tile([P, HW], f16)

                tb = bounds(tidx)
                tcb = taps[cb]
                for c0, c1 in zip(tb[:-1], tb[1:]):
                    PZ = c1 - c0
                    # V-stage: accumulating fp16 diag matmuls, taps at row
                    # offsets dy*56 into guarded X
                    PS = ppool.tile([P, 4 * CHUNK], f32, tag="ps")
                    for ti, dy in enumerate(tcb):
                        for k0 in range(0, PZ, CHUNK):
                            cn = min(CHUNK, PZ - k0)
                            o = W + c0 + k0 + dy * W
                            nc.tensor.matmul(
                                PS[:, k0:k0 + cn],
                                wdt[cb][:, ti * P:(ti + 1) * P],
                                X[:, o:o + cn],
                                start=(ti == 0), stop=(ti == len(tcb) - 1))
                    # drain PSUM -> fp16 VT (ScalarE's only pass)
                    nc.scalar.activation(Vb[:, 1 + c0:1 + c1], PS[:, 0:PZ],
                                         act_copy)
                # PE-H tiles are deferred one iteration so their H
                # matmuls never delay the next tile's V matmuls; DVE-H
                # tiles emit immediately so their stores stream out early
                if pend:
                    emit_h(*pend.pop())
                if tidx in peh:
                    pend.append((tidx, n, cb, Vb, OUT))
                else:
                    emit_h(tidx, n, cb, Vb, OUT)
            if pend:
                emit_h(*pend.pop())
    nc.finalize()
    return nc


def _tap_weights(shift):
    """Per-channel 3-tap weights over offsets {-1,0,1} for shift in [-1,1)."""
    f = np.floor(shift)
    t = (shift - f).astype(np.float32)
    assert np.all((f == -1) | (f == 0)), "shift outside [-1,1) unsupported"
    w_m1 = np.where(f == -1, 1 - t, 0).astype(np.float32)
    w_0 = np.where(f == -1, t, 1 - t).astype(np.float32)
    w_p1 = np.where(f == 0, t, 0).astype(np.float32)
    return w_m1, w_0, w_p1


def _plan(sp):
    """Channel permutation + per-block structure from shift_param."""
    fa = np.floor(sp[:, 0]).astype(np.int64)  # alpha: H shift group
    fb = np.floor(sp[:, 1]).astype(np.int64)  # beta: W shift group
    # sort by (alpha group, beta group); -1 group first
    perm = np.lexsort((fb, fa))
    # if one alpha group can fill a whole 128-block, choose its channel
    # subset so the beta L/R boundary lands on a 32-multiple: the exact
    # per-side shifted adds are then partition-aligned (hardware
    # requirement) and a single merged-weight TMP suffices for that block
    idx = np.arange(C)
    A, B = idx[fa == -1], idx[fa == 0]
    big, small = (A, B) if len(A) >= len(B) else (B, A)
    if len(big) >= P:
        bL = big[fb[big] == -1]
        bR = big[fb[big] == 0]
        kmin = max(0, P - len(bR))
        kmax = min(len(bL), P)
        kL = -1
        k = kmax - kmax % 32
        while k >= kmin:
            kL = k
            break
        if kL >= kmin and kL >= 0:
            blk0 = np.concatenate([bL[:kL], bR[:P - kL]])
            rest = np.concatenate([bL[kL:], bR[P - kL:], small])
            rest = rest[np.lexsort((fb[rest], fa[rest]))]
            perm = np.concatenate([blk0, rest]).astype(np.int64)
    fa_s, fb_s = fa[perm], fb[perm]

    taps = []
    runs = []
    for cb in range(CB):
        cs = slice(cb * P, (cb + 1) * P)
        g = fa_s[cs]
        t = []
        if np.any(g == -1):
            t += [-1, 0]
        if np.any(g == 0):
            if 0 not in t:
                t.append(0)
            t.append(1)
        taps.append(tuple(t))
        sides = (fb_s[cs] == 0).astype(np.int64)  # 0 = left (fb==-1)
        # exact contiguous beta runs
        er = []
        a = 0
        for i in range(1, P + 1):
            if i == P or sides[i] != sides[a]:
                er.append((a, i, int(sides[a])))
                a = i
        if len(er) <= 2 and all(a % 32 == 0 for a, _, _ in er):
            # single-TMP mode: exact per-side ranges, all 32-aligned
            runs.append((1, tuple(er)))
        else:
            # dual-TMP mode: one full-range shifted add per side present;
            # TMP_L/R are exactly zero on other-side channels, so each
            # add is a no-op there.
            r = [(0, P, s) for s in (0, 1) if np.any(sides == s)]
            runs.append((0, tuple(r)))
    return perm, tuple(taps), tuple(runs)


def _host_weights(sp, perm, taps):
    sps = sp[perm]
    wh_m1, wh_0, wh_p1 = _tap_weights(sps[:, 1])  # beta: W shift
    wv_m1, wv_0, wv_p1 = _tap_weights(sps[:, 0])  # alpha: H shift
    vtap = {-1: wv_m1, 0: wv_0, 1: wv_p1}
    NT0 = max(len(t) for t in taps)
    wd = np.zeros((CB, NT0 + 3, P, P), np.float32)
    for cb in range(CB):
        cs = slice(cb * P, (cb + 1) * P)
        for ti, dy in enumerate(taps[cb]):
            wd[cb, ti] = np.diag(vtap[dy][cs])
        for ti, wh in enumerate((wh_m1, wh_0, wh_p1)):
            wd[cb, NT0 + ti] = np.diag(wh[cs])
    wd = wd.transpose(0, 2, 1, 3).reshape(CB, P, (NT0 + 3) * P)
    wd = np.ascontiguousarray(wd.astype(np.float16))
    wvv = np.stack([wh_0, wh_m1, wh_p1, -wh_m1, -wh_p1, wh_m1 + wh_p1],
                   axis=1).astype(np.float32)
    wvv = np.ascontiguousarray(wvv.reshape(CB, P, 6))
    return wd, wvv


def _install_trace_shim():
    """Dev-only: register the NTFF profile hook this container's antenv lacks,
    and stub out the artifact upload (zero-egress container)."""
    import sys
    import types

    try:
        from antenv.axon_hooks import get_axon_ntff_profile_hook  # noqa: F401
    except ImportError:
        from trn_agent_boot.trn_boot import _ntff_profile_via_ctypes

        hook = _ntff_profile_via_ctypes("/opt/axon/libaxon_pjrt.so")
        mod = types.ModuleType("antenv.axon_hooks")
        mod.get_axon_ntff_profile_hook = lambda: hook
        mod.set_axon_ntff_profile_hook = lambda h: None
        import antenv

        sys.modules["antenv.axon_hooks"] = mod
        antenv.axon_hooks = mod

    import concourse.bass_utils as bu

    bu.upload_artifacts = lambda tmpdir: tmpdir


def kernel(x, shift_param):
    from concourse.bass_utils import run_bass_kernel_spmd

    x = np.asarray(x, dtype=np.float32)
    sp = np.asarray(shift_param, dtype=np.float32)
    assert x.shape == (N, C, H, W)

    perm, taps, runs = _plan(sp)
    wd, wvv = _host_weights(sp, perm, taps)
    xp = np.ascontiguousarray(x[:, perm].astype(np.float16))

    npeh = int(os.environ.get("ASL_PEH", "2"))
    nt = NSH * CB
    # spread PE-H tiles mid-stream so their H matmuls overlap DVE H work
    # on neighboring tiles (avoid the fill tiles 0-1 and the drain tail)
    spread = {2: tuple(int(x) for x in os.environ.get('ASL_PEHSET', '5,7').split(',')), 1: (7,), 3: (3, 5, 7), 4: (2, 4, 6, 7)}
    peh = frozenset(spread.get(npeh, range(nt - npeh, nt))) if npeh \
        else frozenset()
    key = (taps, runs, peh, tuple(sorted(PARTROWS.items())))
    if _CACHE.get("key") != key:
        _CACHE["nc"] = _build_nc(taps, runs, peh)
        _CACHE["key"] = key
    nc = _CACHE["nc"]

    in_maps = [{"xs": xp[i * NSH:(i + 1) * NSH], "wd": wd, "wv": wvv}
               for i in range(NCORES)]
    trace = os.environ.get("ASL_TRACE") == "1"
    if trace:
        _install_trace_shim()
    res = run_bass_kernel_spmd(nc, in_maps, list(range(NCORES)), trace=trace)
    if trace:
        print(f"HW exec time: {res.exec_time_ns} ns")
        _CACHE["last_result"] = res
    ysp = np.concatenate([r["ys"] for r in res.results], axis=0)
    out = np.empty((N, C, H, W), np.float32)
    out[:, perm] = ysp.astype(np.float32)
    return out


# revision 41
# speedup vs baseline: 1.0006x; 1.0006x over previous
"""ActiveShiftLayer Trainium2 kernel.

out[n,c,h,w] = bilinear sample of x[n,c, h+alpha_c, w+beta_c], zero outside
the spatial extent.

alpha,beta in [-1,1) => floor in {-1,0}, so the bilinear sample is a
separable 3-tap convolution along H then W with per-channel tap weights;
per channel only 2 of the 3 taps are nonzero in each direction.

Design (measured 63.8-67us HW exec on 8 trn2 cores; v1 baseline was 83.5us):
- fp16 input: x is cast to fp16 on the host, halving HBM read traffic
  (error budget 2e-2 >> fp16's ~1e-4). Output is stored fp16 and upcast on
  the host. ~12.9 MB of DMA per core.
- channel sort: channels are permuted on the host, grouped by floor(alpha)
  (secondary floor(beta)). A 128-channel block whose channels share
  floor(alpha) needs only 2 V-tap matmul passes instead of 3 (the one
  mixed block needs 3).
- V-stage on TensorE: accumulating fp16 diag-stationary matmuls into PSUM
  (2048-col pieces = 4 banks, 512-col chunks); ScalarE drains PSUM -> VT
  (fp16 SBUF, 1-element guards) - its only elementwise pass (~0.96
  ns/elem, the PSUM read rate).
- H-stage for 6 of 8 tiles entirely on VectorE in fp16 fast paths:
  OUT = VT*wh0 and TMP_L/R = VT*wh_m1/p1 are tensor_scalar ops (4x_2p,
  ~0.3 ns/elem); the two shifted accumulates are tensor_tensor adds
  (2x_1p) whose 3D access patterns exclude the out-of-range column.
  TMP_L/R are exactly zero on other-side channels so full-partition-range
  adds are no-ops there. scalar_tensor_tensor was rejected: it supports
  NO DVE perf modes (1 elem/cycle).
- H-stage for tiles {5,7} on TensorE (PE-H): 3 flat-tap diag matmuls over
  the guarded VT; the wrapped edge columns are corrected by tiny strided
  STTs. Tile 7 runs a per-piece lagged fixup+store epilogue to shrink the
  pipeline drain. This balances PE (~38us) against DVE (~41us), the two
  pacing engines.
- The PE-H H-phase is emitted one tile late (software pipelining) so it
  never delays the next tile's V matmuls.
- queues: loads + DVE-tile stores on sync HWDGE (loads are prefetched 4
  tiles ahead so store sem-waits never starve the PE); PE-H piece stores
  on GpSimd SWDGE. Stores must NOT share the Scalar queue: a store
  waiting on DVE head-of-line blocks the next PSUM drain.
- X and VT live in persistent manually-rotated buffers so zero guards are
  memset once; tile 0's H-stage runs in 3 row segments to start the DVE
  stream at ~6us instead of ~10us.

Rejected experiments (measured worse): GpSimd tensor_tensor offload
(Add runs at ~0.42 efficiency), 3+ PE-H tiles (PE overloads), partial
row-split PE-H, ACT-issued center scale, 32-aligned partition-split STT
from PSUM (DVE time depends only on free size, so partition splitting
saves nothing).

The Bass program is built after seeing shift_param (tap sets / partition
runs are data-dependent) and cached by that structure.

Sharding: data-parallel over batch (N=32 -> 4 per core), each core also
splits C=256 into two partition blocks -> 8 tiles of [128 channels
(partitions), 56x56 plane (free dim)] per core. Pure SPMD, no collectives.
"""

import os
import numpy as np

N, C, H, W = 32, 256, 56, 56
NCORES = 8
NSH = N // NCORES  # batches per core
P = 128
CB = C // P        # channel blocks
HW = H * W         # 3136
CHUNK = 512        # one PSUM bank of f32 per matmul
XLEN = W + HW + W + 16  # guard row above/below + pad
# rows of each shifted add handled by Pool (GpSimd) instead of DVE
POOLROWS = int(os.environ.get("ASL_POOLROWS", "0"))
# tile -> rows of its H-stage to run on TensorE (partial PE-H)
PARTROWS = {}
for _kv in os.environ.get("ASL_PARTROWS", "").split(","):
    if _kv and ":" in _kv:
        _k, _v = _kv.split(":")
        if int(_v):
            PARTROWS[int(_k)] = int(_v)

_CACHE = {}


def _build_nc(taps, runs, peh):
    """taps: per-cb tuple of dy offsets needed (subset of (-1,0,1)).
    runs: per-cb tuple of (p0, p1, side) partition ranges; side 0 = left
    tap (out[w] += tmp[w-1]), side 1 = right tap. peh: tile indices whose
    H-stage runs on TensorE (flat taps over guarded VT + wrap fixups)."""
    import concourse.bacc as bacc
    import concourse.mybir as mybir
    import concourse.tile as tile

    f32 = mybir.dt.float32
    f16 = mybir.dt.float16
    add = mybir.AluOpType.add
    mult = mybir.AluOpType.mult
    act_copy = mybir.ActivationFunctionType.Copy

    nc = bacc.Bacc()
    xs = nc.dram_tensor("xs", [NSH, C, H, W], f16, kind="ExternalInput")
    NT0 = max(len(t) for t in taps)
    # V-tap diags followed by 3 H-tap diags (wh_m1, wh_0, wh_p1)
    wd = nc.dram_tensor("wd", [CB, P, (NT0 + 3) * P], f16,
                        kind="ExternalInput")
    # wv[cb] columns: [wh_0, wh_m1, wh_p1, -wh_m1, -wh_p1, wh_m1+wh_p1]
    wv = nc.dram_tensor("wv", [CB, P, 6], f32, kind="ExternalInput")
    ys = nc.dram_tensor("ys", [NSH, C, H, W], f16, kind="ExternalOutput")

    with tile.TileContext(nc) as tc:
        with tc.tile_pool(name="wp", bufs=1) as wp, \
             tc.tile_pool(name="vt", bufs=3) as vpool, \
             tc.tile_pool(name="tm", bufs=4) as tpool, \
             tc.tile_pool(name="op", bufs=4) as opool, \
             tc.tile_pool(name="ps", bufs=2, space="PSUM") as ppool:

            wdt = []
            wvt = []

            def load_weights(cb):
                t = wp.tile([P, (NT0 + 3) * P], f16, tag=f"wd{cb}")
                nc.sync.dma_start(t[:], wd[cb])
                wdt.append(t)
                v = wp.tile([P, 6], f32, tag=f"wv{cb}")
                nc.sync.dma_start(v[:], wv[cb])
                wvt.append(v)

            # persistent X buffers: guards zeroed once, loads only rewrite
            # the middle region
            NXBUF = 5
            xbufs = []
            for i in range(NXBUF):
                xb = wp.tile([P, XLEN], f16, tag=f"X{i}")
                nc.vector.memset(xb[:, 0:W], 0.0)
                nc.vector.memset(xb[:, W + HW:W + HW + W], 0.0)
                xbufs.append(xb)
            # persistent VT buffers with 1-element guards at 0 and 1+HW
            # for the PE H-stage flat taps
            NVBUF = 4
            vbufs = []
            for i in range(NVBUF):
                vb = wp.tile([P, 3200], f16, tag=f"V{i}")
                nc.vector.memset(vb[:, 0:1], 0.0)
                nc.vector.memset(vb[:, 1 + HW:2 + HW], 0.0)
                vbufs.append(vb)

            load_weights(0)

            tiles = [(n, cb) for n in range(NSH) for cb in range(CB)]
            NT = len(tiles)

            # column boundaries of PSUM pieces per tile (<= 2048 cols each)
            def bounds(idx):
                if idx == 0:
                    return [0, 512, 1024, 2048, 3136]
                if idx == NT - 1:
                    # row-aligned so the PE-H epilogue can run per piece
                    return [0, 1008, 2016, 2576, 3136]
                return [0, 2048, 3136]

            def issue_load(idx):
                # segmented so piece i only depends on segments 0..i
                ln, lcb = tiles[idx]
                lcs = slice(lcb * P, (lcb + 1) * P)
                X = xbufs[idx % NXBUF]
                xflat = xs[ln, lcs, :, :].rearrange("p h w -> p (h w)")
                b = bounds(idx)
                cuts = [min(-(-c1 // W) + 1, H) for c1 in b[1:-1]] + [H]
                r0 = 0
                for r1 in cuts:
                    if r1 > r0:
                        nc.sync.dma_start(X[:, W + r0 * W:W + r1 * W],
                                          xflat[:, r0 * W:r1 * W])
                    r0 = r1
                return X

            xtiles = {}
            xtiles[0] = issue_load(0)
            load_weights(1)
            xtiles[1] = issue_load(1)
            xtiles[2] = issue_load(2)
            xtiles[3] = issue_load(3)

            pend = []  # (tidx, n, cb, Vb, OUT) awaiting H-phase

            def emit_h(tidx, n, cb, Vb, OUT):
                wvc = wvt[cb]
                cs = slice(cb * P, (cb + 1) * P)
                VT = Vb[:, 1:1 + HW]
                tb = bounds(tidx)
                o3 = OUT[:].rearrange("p (h w) -> p h w", w=W)
                hsegs = ([(0, 9), (9, 28), (28, 56)] if tidx == 0 else
                         [(0, 56)])
                if tidx in peh:
                    # H-stage on TensorE: 3 flat taps over guarded VT;
                    # wrapped columns corrected by two tiny STTs after.
                    # For the final tile the fixup+store epilogue runs per
                    # piece (lagged one piece so the vg55 read of the next
                    # row's vt is drained), shrinking the pipeline tail.
                    vg0 = Vb[:, 0:HW].rearrange(
                        "p (h w) -> p h w", w=W)[:, :, 0]
                    vg55 = Vb[:, 57:57 + HW].rearrange(
                        "p (h w) -> p h w", w=W)[:, :, 0]
                    yflatp = ys[n, cs, :, :].rearrange("p h w -> p (h w)")

                    def fix_store(r0, r1):
                        nc.vector.scalar_tensor_tensor(
                            o3[:, r0:r1, 0], vg0[:, r0:r1], wvc[:, 3:4],
                            o3[:, r0:r1, 0], op0=mult, op1=add)
                        nc.vector.scalar_tensor_tensor(
                            o3[:, r0:r1, W - 1], vg55[:, r0:r1],
                            wvc[:, 4:5], o3[:, r0:r1, W - 1],
                            op0=mult, op1=add)
                        nc.gpsimd.dma_start(yflatp[:, r0 * W:r1 * W],
                                            OUT[:, r0 * W:r1 * W])

                    lagged = tidx == NT - 1
                    prev = None
                    for c0, c1 in zip(tb[:-1], tb[1:]):
                        PZ = c1 - c0
                        PS2 = ppool.tile([P, 4 * CHUNK], f32, tag="ps")
                        for ti in range(3):
                            for k0 in range(0, PZ, CHUNK):
                                cn = min(CHUNK, PZ - k0)
                                o = c0 + k0 + ti  # dx = ti - 1, +1 guard
                                nc.tensor.matmul(
                                    PS2[:, k0:k0 + cn],
                                    wdt[cb][:, (NT0 + ti) * P:
                                             (NT0 + ti + 1) * P],
                                    Vb[:, o:o + cn],
                                    start=(ti == 0), stop=(ti == 2))
                        nc.scalar.activation(OUT[:, c0:c1], PS2[:, 0:PZ],
                                             act_copy)
                        if lagged:
                            if prev is not None:
                                fix_store(prev[0] // W, prev[1] // W)
                            prev = (c0, c1)
                    if lagged:
                        fix_store(prev[0] // W, prev[1] // W)
                    else:
                        fix_store(0, H)
                    return
                # H-stage on DVE in fp16 fast paths (tensor_scalar at
                # 4x_2p, shifted tensor_tensor adds at 2x_1p).
                # Separate prescaled planes per side: TMP_L/TMP_R are
                # exactly zero on other-side channels, so each
                # full-range shifted add is a no-op there.
                actc = tidx in (2, 4) and int(
                    os.environ.get("ASL_ACTC", "0"))
                # partial PE-H: rows [0, pr) of this tile's H-stage run on
                # TensorE (flat taps + fixups), the rest stays on DVE
                pr = PARTROWS.get(tidx, 0)
                if pr:
                    pcc = pr * W
                    vg0 = Vb[:, 0:HW].rearrange(
                        "p (h w) -> p h w", w=W)[:, :, 0]
                    vg55 = Vb[:, 57:57 + HW].rearrange(
                        "p (h w) -> p h w", w=W)[:, :, 0]
                    yflatp = ys[n, cs, :, :].rearrange("p h w -> p (h w)")
                    PS2 = ppool.tile([P, 4 * CHUNK], f32, tag="ps")
                    for ti in range(3):
                        for k0 in range(0, pcc, CHUNK):
                            cn = min(CHUNK, pcc - k0)
                            nc.tensor.matmul(
                                PS2[:, k0:k0 + cn],
                                wdt[cb][:, (NT0 + ti) * P:
                                         (NT0 + ti + 1) * P],
                                Vb[:, k0 + ti:k0 + ti + cn],
                                start=(ti == 0), stop=(ti == 2))
                    nc.scalar.activation(OUT[:, 0:pcc], PS2[:, 0:pcc],
                                         act_copy)
                    nc.vector.scalar_tensor_tensor(
                        o3[:, 0:pr, 0], vg0[:, 0:pr], wvc[:, 3:4],
                        o3[:, 0:pr, 0], op0=mult, op1=add)
                    nc.vector.scalar_tensor_tensor(
                        o3[:, 0:pr, W - 1], vg55[:, 0:pr],
                        wvc[:, 4:5], o3[:, 0:pr, W - 1],
                        op0=mult, op1=add)
                    nc.gpsimd.dma_start(yflatp[:, 0:pcc], OUT[:, 0:pcc])
                    hsegs = [(pr, 56)]
                for (r0, r1) in hsegs:
                    q0, q1 = r0 * W, r1 * W
                    if actc:
                        # offload the center scale to ScalarE (has slack)
                        nc.scalar.activation(OUT[:, q0:q1], VT[:, q0:q1],
                                             act_copy, scale=wvc[:, 0:1])
                    else:
                        nc.vector.tensor_scalar_mul(
                            OUT[:, q0:q1], VT[:, q0:q1], wvc[:, 0:1])
                    single, rblk = runs[cb]
                    t3s = {}
                    if single:
                        # one TMP with the per-channel nonzero outer
                        # weight; the adds use exact partition runs
                        TMP = tpool.tile([P, HW], f16, tag="tmp0")
                        nc.vector.tensor_scalar_mul(
                            TMP[:, q0:q1], VT[:, q0:q1], wvc[:, 5:6])
                        t3s[0] = t3s[1] = TMP[:].rearrange(
                            "p (h w) -> p h w", w=W)
                    else:
                        for (_, _, s) in rblk:
                            TMP = tpool.tile([P, HW], f16, tag=f"tmp{s}")
                            nc.vector.tensor_scalar_mul(
                                TMP[:, q0:q1], VT[:, q0:q1],
                                wvc[:, 1 + s:2 + s])
                            t3s[s] = TMP[:].rearrange(
                                "p (h w) -> p h w", w=W)
                    hs = min(POOLROWS, r1 - r0) if r0 == 0 else 0
                    for (a, b2, side) in rblk:
                        t3 = t3s[side]
                        if side == 0:
                            if hs:
                                nc.gpsimd.tensor_tensor(
                                    o3[a:b2, r0:r0 + hs, 1:W],
                                    t3[a:b2, r0:r0 + hs, 0:W - 1],
                                    o3[a:b2, r0:r0 + hs, 1:W], op=add)
                            nc.vector.tensor_tensor(
                                o3[a:b2, r0 + hs:r1, 1:W],
                                t3[a:b2, r0 + hs:r1, 0:W - 1],
                                o3[a:b2, r0 + hs:r1, 1:W], op=add)
                        else:
                            if hs:
                                nc.gpsimd.tensor_tensor(
                                    o3[a:b2, r0:r0 + hs, 0:W - 1],
                                    t3[a:b2, r0:r0 + hs, 1:W],
                                    o3[a:b2, r0:r0 + hs, 0:W - 1], op=add)
                            nc.vector.tensor_tensor(
                                o3[a:b2, r0 + hs:r1, 0:W - 1],
                                t3[a:b2, r0 + hs:r1, 1:W],
                                o3[a:b2, r0 + hs:r1, 0:W - 1], op=add)
                    yflat = ys[n, cs, :, :].rearrange("p h w -> p (h w)")
                    nc.sync.dma_start(yflat[:, q0:q1], OUT[:, q0:q1])

            for tidx, (n, cb) in enumerate(tiles):
                cs = slice(cb * P, (cb + 1) * P)
                if tidx + 4 < NT:
                    xtiles[tidx + 4] = issue_load(tidx + 4)
                X = xtiles.pop(tidx)

                Vb = vbufs[tidx % NVBUF]
                OUT = opool.tile([P, HW], f16)

                tb = bounds(tidx)
                tcb = taps[cb]
                for c0, c1 in zip(tb[:-1], tb[1:]):
                    PZ = c1 - c0
                    # V-stage: accumulating fp16 diag matmuls, taps at row
                    # offsets dy*56 into guarded X
                    PS = ppool.tile([P, 4 * CHUNK], f32, tag="ps")
                    for ti, dy in enumerate(tcb):
                        for k0 in range(0, PZ, CHUNK):
                            cn = min(CHUNK, PZ - k0)
                            o = W + c0 + k0 + dy * W
                            nc.tensor.matmul(
                                PS[:, k0:k0 + cn],
                                wdt[cb][:, ti * P:(ti + 1) * P],
                                X[:, o:o + cn],
                                start=(ti == 0), stop=(ti == len(tcb) - 1))
                    # drain PSUM -> fp16 VT (ScalarE's only pass)
                    nc.scalar.activation(Vb[:, 1 + c0:1 + c1], PS[:, 0:PZ],
                                         act_copy)
                # PE-H tiles are deferred one iteration so their H
                # matmuls never delay the next tile's V matmuls; DVE-H
                # tiles emit immediately so their stores stream out early
                if pend:
                    emit_h(*pend.pop())
                if tidx in peh:
                    pend.append((tidx, n, cb, Vb, OUT))
                else:
                    emit_h(tidx, n, cb, Vb, OUT)
            if pend:
                emit_h(*pend.pop())
    nc.finalize()
    return nc


def _tap_weights(shift):
    """Per-channel 3-tap weights over offsets {-1,0,1} for shift in [-1,1)."""
    f = np.floor(shift)
    t = (shift - f).astype(np.float32)
    assert np.all((f == -1) | (f == 0)), "shift outside [-1,1) unsupported"
    w_m1 = np.where(f == -1, 1 - t, 0).astype(np.float32)
    w_0 = np.where(f == -1, t, 1 - t).astype(np.float32)
    w_p1 = np.where(f == 0, t, 0).astype(np.float32)
    return w_m1, w_0, w_p1


def _plan(sp):
    """Channel permutation + per-block structure from shift_param."""
    fa = np.floor(sp[:, 0]).astype(np.int64)  # alpha: H shift group
    fb = np.floor(sp[:, 1]).astype(np.int64)  # beta: W shift group
    # sort by (alpha group, beta group); -1 group first
    perm = np.lexsort((fb, fa))
    # if one alpha group can fill a whole 128-block, choose its channel
    # subset so the beta L/R boundary lands on a 32-multiple: the exact
    # per-side shifted adds are then partition-aligned (hardware
    # requirement) and a single merged-weight TMP suffices for that block
    idx = np.arange(C)
    A, B = idx[fa == -1], idx[fa == 0]
    big, small = (A, B) if len(A) >= len(B) else (B, A)
    if len(big) >= P:
        bL = big[fb[big] == -1]
        bR = big[fb[big] == 0]
        kmin = max(0, P - len(bR))
        kmax = min(len(bL), P)
        kL = -1
        k = kmax - kmax % 32
        while k >= kmin:
            kL = k
            break
        if kL >= kmin and kL >= 0:
            blk0 = np.concatenate([bL[:kL], bR[:P - kL]])
            rest = np.concatenate([bL[kL:], bR[P - kL:], small])
            rest = rest[np.lexsort((fb[rest], fa[rest]))]
            perm = np.concatenate([blk0, rest]).astype(np.int64)
    fa_s, fb_s = fa[perm], fb[perm]

    taps = []
    runs = []
    for cb in range(CB):
        cs = slice(cb * P, (cb + 1) * P)
        g = fa_s[cs]
        t = []
        if np.any(g == -1):
            t += [-1, 0]
        if np.any(g == 0):
            if 0 not in t:
                t.append(0)
            t.append(1)
        taps.append(tuple(t))
        sides = (fb_s[cs] == 0).astype(np.int64)  # 0 = left (fb==-1)
        # exact contiguous beta runs
        er = []
        a = 0
        for i in range(1, P + 1):
            if i == P or sides[i] != sides[a]:
                er.append((a, i, int(sides[a])))
                a = i
        if len(er) == 1:
            # single-TMP mode: one pure full-range run only. NOTE:
            # 32/64-aligned partition-restricted tt adds are LEGAL but
            # measured slower than full-range dual-TMP (partition-
            # restricted DVE ops appear to lose fast bank routing), so
            # two aligned sub-runs are NOT taken here.
            runs.append((1, tuple(er)))
        else:
            # dual-TMP mode: one full-range shifted add per side present;
            # TMP_L/R are exactly zero on other-side channels, so each
            # add is a no-op there.
            r = [(0, P, s) for s in (0, 1) if np.any(sides == s)]
            runs.append((0, tuple(r)))
    return perm, tuple(taps), tuple(runs)


def _host_weights(sp, perm, taps):
    sps = sp[perm]
    wh_m1, wh_0, wh_p1 = _tap_weights(sps[:, 1])  # beta: W shift
    wv_m1, wv_0, wv_p1 = _tap_weights(sps[:, 0])  # alpha: H shift
    vtap = {-1: wv_m1, 0: wv_0, 1: wv_p1}
    NT0 = max(len(t) for t in taps)
    wd = np.zeros((CB, NT0 + 3, P, P), np.float32)
    for cb in range(CB):
        cs = slice(cb * P, (cb + 1) * P)
        for ti, dy in enumerate(taps[cb]):
            wd[cb, ti] = np.diag(vtap[dy][cs])
        for ti, wh in enumerate((wh_m1, wh_0, wh_p1)):
            wd[cb, NT0 + ti] = np.diag(wh[cs])
    wd = wd.transpose(0, 2, 1, 3).reshape(CB, P, (NT0 + 3) * P)
    wd = np.ascontiguousarray(wd.astype(np.float16))
    wvv = np.stack([wh_0, wh_m1, wh_p1, -wh_m1, -wh_p1, wh_m1 + wh_p1],
                   axis=1).astype(np.float32)
    wvv = np.ascontiguousarray(wvv.reshape(CB, P, 6))
    return wd, wvv


def _install_trace_shim():
    """Dev-only: register the NTFF profile hook this container's antenv lacks,
    and stub out the artifact upload (zero-egress container)."""
    import sys
    import types

    try:
        from antenv.axon_hooks import get_axon_ntff_profile_hook  # noqa: F401
    except ImportError:
        from trn_agent_boot.trn_boot import _ntff_profile_via_ctypes

        hook = _ntff_profile_via_ctypes("/opt/axon/libaxon_pjrt.so")
        mod = types.ModuleType("antenv.axon_hooks")
        mod.get_axon_ntff_profile_hook = lambda: hook
        mod.set_axon_ntff_profile_hook = lambda h: None
        import antenv

        sys.modules["antenv.axon_hooks"] = mod
        antenv.axon_hooks = mod

    import concourse.bass_utils as bu

    bu.upload_artifacts = lambda tmpdir: tmpdir


def kernel(x, shift_param):
    from concourse.bass_utils import run_bass_kernel_spmd

    x = np.asarray(x, dtype=np.float32)
    sp = np.asarray(shift_param, dtype=np.float32)
    assert x.shape == (N, C, H, W)

    perm, taps, runs = _plan(sp)
    wd, wvv = _host_weights(sp, perm, taps)
    xp = np.ascontiguousarray(x[:, perm].astype(np.float16))

    npeh = int(os.environ.get("ASL_PEH", "2"))
    nt = NSH * CB
    # spread PE-H tiles mid-stream so their H matmuls overlap DVE H work
    # on neighboring tiles (avoid the fill tiles 0-1 and the drain tail)
    spread = {2: tuple(int(x) for x in os.environ.get('ASL_PEHSET', '5,7').split(',')), 1: (7,), 3: (3, 5, 7), 4: (2, 4, 6, 7)}
    peh = frozenset(spread.get(npeh, range(nt - npeh, nt))) if npeh \
        else frozenset()
    key = (taps, runs, peh, tuple(sorted(PARTROWS.items())))
    if _CACHE.get("key") != key:
        _CACHE["nc"] = _build_nc(taps, runs, peh)
        _CACHE["key"] = key
    nc = _CACHE["nc"]

    in_maps = [{"xs": xp[i * NSH:(i + 1) * NSH], "wd": wd, "wv": wvv}
               for i in range(NCORES)]
    trace = os.environ.get("ASL_TRACE") == "1"
    if trace:
        _install_trace_shim()
    res = run_bass_kernel_spmd(nc, in_maps, list(range(NCORES)), trace=trace)
    if trace:
        print(f"HW exec time: {res.exec_time_ns} ns")
        _CACHE["last_result"] = res
    ysp = np.concatenate([r["ys"] for r in res.results], axis=0)
    out = np.empty((N, C, H, W), np.float32)
    out[:, perm] = ysp.astype(np.float32)
    return out
